# revision 1
# baseline (speedup 1.0000x reference)
"""Causal self-attention Bass/TRN2 kernel for nn_CausalSelfAttention.

Shapes (hardcoded): query [2, 2048, 1024], 16 heads, d=64.
Sharding: 8 cores = 2 batches x 4 head-groups (4 heads per core, tensor
parallel on QKV/proj weight columns). Each core computes a partial output
projection out_t = Wp_slice^T @ y^T (shape [1024, 2048]); host sums the 4
partials per batch, transposes, and adds bp.

Per-core pipeline:
  1. PE-transpose X [2048,1024] -> X^T [1024,2048] tiles (fp32 identity mm)
  2. Q^T, K^T = Wq/Wk_slice^T @ X^T (+bias via ACT copy), [256, 2048] f32r
     V = X @ Wv_slice (+bias via K=1 ones matmul), [2048, 256] f32r
  3. Per head-pair: S^T_j = k^T_j.T-style nc_matmul(kT chunk, qT), row-packed
     2 heads via tile_position (0,0)/(64,0); additive -1e30 triangle mask on
     diagonal 128-blocks; ACT exp (scale=1/8, no max-subtraction -- scores
     are bounded |s|<9 for this problem) -> P_j f32r; PV + denominator
     (ones-matmul) col-packed via tile_position (0,0)/(0,64); per-head
     normalization y^T *= 1/den fused on DVE.
  4. out_t = Wp_slice^T @ y^T.

This walrus build accepts only ONE sync-wait command per TPB instruction, so
after Tile scheduling we hoist excess waits into standalone InstEventSemaphore
instructions (split_excess_waits).
"""

import numpy as np

import concourse.bass as bass
import concourse.mybir as mybir
import concourse.tile as tile
from concourse.bass_utils import run_bass_kernel_spmd

B, T, C, H = 2, 2048, 1024, 16
D = C // H            # 64 head dim
HC = 4                # heads per core
DC = HC * D           # 256 dcols per core
KT = C // 128         # 8 contraction tiles
NT = T // 128         # 16 t-tiles
TCH = T // 512        # 4 t-chunks of 512
SCALE = 1.0 / np.sqrt(D)
NEG = -1.0e30

f32 = mybir.dt.float32
f32r = mybir.dt.float32r

_CACHE = {}


def _split_excess_waits(nc, max_inline=1):
    """Hoist excess per-instruction waits into standalone event-sem waits."""
    n = 0
    for f in nc.m.functions:
        for bb in f.blocks:
            new_insts = []
            for inst in bb.instructions:
                si = inst.sync_info
                waits = list(si.on_wait) if (si is not None and si.on_wait) else []
                if len(waits) > max_inline:
                    hoist, keep = waits[:-max_inline], waits[-max_inline:]
                    for w in hoist:
                        ev = mybir.InstEventSemaphore(
                            name=nc.get_next_instruction_name(),
                            engine=inst.engine,
                            ins=[],
                            outs=[],
                            sync_info=mybir.SyncInfo(on_wait=[w], on_update=[]),
                        )
                        nc.register_instruction(ev, overwrite=True)
                        new_insts.append(ev)
                        n += 1
                    si.on_wait = keep
                new_insts.append(inst)
            bb.instructions[:] = new_insts
    return n


def _make_identity(nc, ident):
    # affine_select KEEPS in_ where the predicate holds and writes `fill`
    # where it does not: identity = fill 1.0 where NOT (p - f != 0).
    nc.gpsimd.memset(ident, 0.0)
    nc.gpsimd.affine_select(
        out=ident, in_=ident, compare_op=mybir.AluOpType.not_equal,
        fill=1.0, base=0, pattern=[[-1, 128]], channel_multiplier=1,
    )


def _make_diag_mask(nc, mask):
    """mask[p, f] = 0 where f >= p (valid, t>=s) else -1e30."""
    nc.gpsimd.memset(mask, 0.0)
    nc.gpsimd.affine_select(
        out=mask, in_=mask, compare_op=mybir.AluOpType.is_ge,
        fill=NEG, base=0, pattern=[[1, 128]], channel_multiplier=-1,
    )


def _build_program(debug_dumps=False, stages=4):
    import os as _os
    skip_v = bool(_os.environ.get("SKIP_V"))
    skip_k = bool(_os.environ.get("SKIP_K"))
    skip_q = bool(_os.environ.get("SKIP_Q"))
    nc = bass.Bass("TRN2", target_bir_lowering=False, debug=False)

    x_d = nc.dram_tensor("x", [T, C], f32, kind="ExternalInput").ap()
    wq_d = nc.dram_tensor("wq", [C, DC], f32r, kind="ExternalInput").ap()
    wk_d = nc.dram_tensor("wk", [C, DC], f32r, kind="ExternalInput").ap()
    wv_d = nc.dram_tensor("wv", [C, DC], f32r, kind="ExternalInput").ap()
    wp_d = nc.dram_tensor("wp", [DC, C], f32r, kind="ExternalInput").ap()
    bq_d = nc.dram_tensor("bq", [DC], f32, kind="ExternalInput").ap()
    bk_d = nc.dram_tensor("bk", [DC], f32, kind="ExternalInput").ap()
    bv_d = nc.dram_tensor("bv", [1, DC], f32r, kind="ExternalInput").ap()
    ones_d = nc.dram_tensor("ones_pv", [128, 64], f32r, kind="ExternalInput").ap()
    onesrow_d = nc.dram_tensor("onesrow", [1, 128], f32r, kind="ExternalInput").ap()
    out_d = nc.dram_tensor("out_t", [C, T], f32, kind="ExternalOutput").ap()

    with (
        tile.TileContext(nc) as tc,
        nc.allow_low_precision("float32r is 32-bit storage; rounding is benign"),
    ):
        with (
            tc.tile_pool(name="const", bufs=1) as cpool,
            tc.tile_pool(name="big", bufs=1) as big,
        ):
            # ---- constants ----
            ident = cpool.tile([128, 128], f32)
            _make_identity(nc, ident)
            dmask = cpool.tile([128, 128], f32)
            _make_diag_mask(nc, dmask)
            bq_sb = cpool.tile([128, 2, 1], f32)
            bk_sb = cpool.tile([128, 2, 1], f32)
            for m in range(2):
                nc.sync.dma_start(
                    out=bq_sb[:, m, :],
                    in_=bq_d[bass.ds(128 * m, 128)].rearrange("(p o) -> p o", o=1),
                )
                nc.sync.dma_start(
                    out=bk_sb[:, m, :],
                    in_=bk_d[bass.ds(128 * m, 128)].rearrange("(p o) -> p o", o=1),
                )
            bv_sb = cpool.tile([1, DC], f32r)
            nc.sync.dma_start(out=bv_sb, in_=bv_d)
            ones_pv = cpool.tile([128, 64], f32r)
            nc.sync.dma_start(out=ones_pv, in_=ones_d)
            onesrow = cpool.tile([1, 128], f32r)
            nc.sync.dma_start(out=onesrow, in_=onesrow_d)

            # ---- persistent big tensors ----
            qt = big.tile([128, 2, T], f32r)   # Q^T  [dcol, t]
            kt = big.tile([128, 2, T], f32r)   # K^T
            # V augmented per head: [s, 65] = [V_h | ones]; M=65 PV matmul
            # then computes y rows 0..63 and the softmax denominator row 64.
            va = big.tile([128, HC, NT, 65], f32r)
            yt = big.tile([128, 2, T], f32r)   # normalized y^T

            # ================= stage 1+2: transpose + projections ==========
            with (
                tc.tile_pool(name="xtp", bufs=1) as xtp,
                tc.tile_pool(name="wqk", bufs=1) as wqk,
                tc.tile_pool(name="xn_p", bufs=3) as xn_p,
                tc.tile_pool(name="ps_t", bufs=2, space="PSUM") as ps_t,
                tc.tile_pool(name="ps_qk", bufs=2, space="PSUM") as ps_qk,
                tc.tile_pool(name="ps_v", bufs=2, space="PSUM") as ps_v,
            ):
                xt = xtp.tile([128, KT, T], f32r)  # X^T
                wq_sb = wqk.tile([128, KT, DC], f32r)
                wk_sb = wqk.tile([128, KT, DC], f32r)
                wv_sb = wqk.tile([128, KT, DC], f32r)
                for k in range(KT):
                    nc.sync.dma_start(out=wq_sb[:, k, :], in_=wq_d[bass.ts(k, 128), :])
                    nc.sync.dma_start(out=wk_sb[:, k, :], in_=wk_d[bass.ts(k, 128), :])
                    nc.sync.dma_start(out=wv_sb[:, k, :], in_=wv_d[bass.ts(k, 128), :])

                # transpose X -> X^T; batch 4 transposes per full PSUM bank
                # so no engine ever reads a bank the PE is still writing
                xn_o = None
                if debug_dumps:
                    xn_o = nc.dram_tensor(
                        "xn_o", [128, C], f32, kind="ExternalOutput").ap()
                for it in range(NT):
                    xn = xn_p.tile([128, C], f32)
                    nc.sync.dma_start(out=xn, in_=x_d[bass.ts(it, 128), :])
                    if debug_dumps and it == 0:
                        nc.sync.dma_start(out=xn_o, in_=xn)
                    for kb in range(KT // 4):
                        tp = ps_t.tile([128, 512], f32)
                        for kk in range(4):
                            k = 4 * kb + kk
                            nc.tensor.transpose(
                                tp[:, bass.ts(kk, 128)], xn[:, bass.ts(k, 128)],
                                ident,
                            )
                        nc.vector.tensor_copy(
                            out=xt[:, 4 * kb:4 * kb + 4, bass.ts(it, 128)],
                            in_=tp.rearrange("p (k t) -> p k t", k=4),
                        )

                # Q^T / K^T projections (+bias via ACT copy)
                for m in range(2 if not skip_q else 0):
                    for g in range(TCH):
                        qp = ps_qk.tile([128, 512], f32)
                        for k in range(KT):
                            nc.tensor.matmul(
                                qp,
                                wq_sb[:, k, bass.ts(m, 128)],
                                xt[:, k, bass.ts(g, 512)],
                                start=(k == 0), stop=(k == KT - 1),
                            )
                        nc.scalar.activation(
                            out=qt[:, m, bass.ts(g, 512)], in_=qp,
                            func=mybir.ActivationFunctionType.Identity,
                            bias=bq_sb[:, m, :], scale=1.0,
                        )
                        kp = ps_qk.tile([128, 512], f32)
                        for k in range(KT if not skip_k else 0):
                            nc.tensor.matmul(
                                kp,
                                wk_sb[:, k, bass.ts(m, 128)],
                                xt[:, k, bass.ts(g, 512)],
                                start=(k == 0), stop=(k == KT - 1),
                            )
                        if not skip_k:
                            nc.scalar.activation(
                                out=kt[:, m, bass.ts(g, 512)], in_=kp,
                                func=mybir.ActivationFunctionType.Identity,
                                bias=bk_sb[:, m, :], scale=1.0,
                            )

                # V natural (+bias via K=1 ones matmul)
                if debug_dumps:
                    xt_o = nc.dram_tensor(
                        "xt_o", [128, KT, T], f32, kind="ExternalOutput").ap()
                    wq_o = nc.dram_tensor(
                        "wq_o", [128, KT, DC], f32, kind="ExternalOutput").ap()
                    nc.sync.dma_start(out=xt_o, in_=xt.bitcast(f32))
                    nc.sync.dma_start(out=wq_o, in_=wq_sb.bitcast(f32))

                for it in range(NT if not skip_v else 0):
                    # full-bank allocation (use first DC cols) to avoid
                    # intra-bank PE-write / DVE-read overlap
                    vp_full = ps_v.tile([128, 512], f32)
                    vp = vp_full[:, 0:DC]
                    for k in range(KT):
                        nc.tensor.matmul(
                            vp,
                            xt[:, k, bass.ts(it, 128)],
                            wv_sb[:, k, :],
                            start=(k == 0), stop=False,
                        )
                    import os as _os
                    if not _os.environ.get("SKIP_BV"):
                        nc.tensor.matmul(
                            vp, onesrow, bv_sb, start=False, stop=True,
                        )
                    else:
                        pass
                    for h in range(HC):
                        nc.vector.tensor_copy(
                            out=va[:, h, it, 0:64], in_=vp[:, bass.ts(h, 64)]
                        )
                # ones column of each v_aug
                for h in range(HC):
                    nc.vector.tensor_copy(
                        out=va[:, h, :, 64:65],
                        in_=ones_pv[:, 0:NT].rearrange("p (n o) -> p n o", o=1),
                    )

            # ================= stage 3: attention =========================
            def attention_headpair(hp, pools, after_g=None):
                pp, den_p, ps_s, ps_y, ps_b = pools  # ps_b aliases ps_o
                h1, h2 = 2 * hp, 2 * hp + 1
                for g in range(TCH):
                    yd1 = ps_y.tile([128, 512], f32, name="yd1")
                    yd2 = ps_y.tile([128, 512], f32, name="yd2")
                    nj = 4 * g + 4
                    for j in range(nj):
                        r = j - 4 * g
                        lo = 128 * r if r > 0 else 0
                        w = 512 - lo
                        # both heads' S^T in one 2-bank psum tile
                        s12 = ps_s.tile([128, 1024], f32, name="s12")
                        tsl = bass.ds(512 * g + lo, w)
                        nc.tensor.matmul(
                            s12[:, lo:512], kt[0:64, hp, bass.ts(j, 128)],
                            qt[0:64, hp, tsl], start=True, stop=True,
                        )
                        nc.tensor.matmul(
                            s12[:, 512 + lo:1024], kt[64:128, hp, bass.ts(j, 128)],
                            qt[64:128, hp, tsl], start=True, stop=True,
                        )
                        if r >= 0:
                            nc.vector.tensor_add(
                                s12[:, lo:lo + 128], s12[:, lo:lo + 128], dmask
                            )
                            nc.vector.tensor_add(
                                s12[:, 512 + lo:512 + lo + 128],
                                s12[:, 512 + lo:512 + lo + 128], dmask
                            )
                        p12 = pp.tile([128, 1024], f32r, name="p12")
                        sv = s12.rearrange("p (h t) -> p h t", h=2)[:, :, lo:]
                        pv = p12.rearrange("p (h t) -> p h t", h=2)[:, :, lo:]
                        nc.scalar.activation(
                            out=pv, in_=sv,
                            func=mybir.ActivationFunctionType.Exp,
                            scale=float(SCALE),
                        )
                        last = j == nj - 1
                        nc.tensor.matmul(
                            yd1[0:65, lo:], va[:, h1 % 4, j, :],
                            p12[:, lo:512], start=(j == 0), stop=last,
                            skip_group_check=True,
                        )
                        nc.tensor.matmul(
                            yd2[0:65, lo:], va[:, h2 % 4, j, :],
                            p12[:, 512 + lo:1024], start=(j == 0), stop=last,
                            skip_group_check=True,
                        )
                    # normalize: recip of den row 64, broadcast to 64 rows
                    # via ones matmul, multiply into y rows
                    for odd, yd in ((0, yd1), (1, yd2)):
                        r1 = den_p.tile([128, 512], f32r, name="r1")
                        nc.vector.reciprocal(
                            out=r1[64:65, :], in_=yd[64:65, :]
                        )
                        # K=1 matmul with lhsT/rhs at partition 64 (row
                        # group (64,0)): broadcasts 1/den to 64 rows without
                        # a partition-move DMA in the critical chain
                        bc = ps_b.tile([128, 512], f32, name="op")[0:64, :]
                        nc.tensor.matmul(
                            bc, ones_pv[64:65, :], r1[64:65, :],
                            start=True, stop=True,
                        )
                        rb = den_p.tile([64, 512], f32, name="rb")
                        nc.vector.tensor_copy(out=rb, in_=bc)
                        if odd == 0:
                            nc.vector.tensor_mul(
                                yt[0:64, hp, bass.ts(g, 512)], yd[0:64, :], rb
                            )
                        else:
                            ytmp = den_p.tile([64, 512], f32r, name="ytmp")
                            nc.vector.tensor_mul(ytmp, yd[0:64, :], rb)
                            nc.sync.dma_start(
                                out=yt[64:128, hp, bass.ts(g, 512)], in_=ytmp,
                            )
                    if after_g is not None:
                        after_g(g)

            if stages >= 3:
                with (
                    tc.tile_pool(name="pp", bufs=4) as pp,
                    tc.tile_pool(name="den_p", bufs=2) as den_p,
                    tc.tile_pool(name="wpp", bufs=1) as wpp,
                    tc.tile_pool(name="ob_p", bufs=3) as ob_p,
                    tc.tile_pool(name="ps_s", bufs=2, space="PSUM") as ps_s,
                    tc.tile_pool(name="ps_y", bufs=1, space="PSUM") as ps_y,
                    tc.tile_pool(name="ps_o", bufs=2, space="PSUM") as ps_o,
                ):
                    wp_sb = wpp.tile([128, 2, 8, 128], f32r)
                    for m in range(2):
                        for mo in range(8):
                            nc.sync.dma_start(
                                out=wp_sb[:, m, mo, :],
                                in_=wp_d[bass.ts(m, 128), bass.ts(mo, 128)],
                            )

                    def outproj_g(g):
                        for mo in range(8):
                            op = ps_o.tile([128, 512], f32, name="op")
                            for m in range(2):
                                nc.tensor.matmul(
                                    op, wp_sb[:, m, mo, :],
                                    yt[:, m, bass.ts(g, 512)],
                                    start=(m == 0), stop=(m == 1),
                                )
                            ob = ob_p.tile([128, 512], f32, name="ob")
                            nc.vector.tensor_copy(out=ob, in_=op)
                            nc.sync.dma_start(
                                out=out_d[bass.ts(mo, 128), bass.ts(g, 512)],
                                in_=ob,
                            )

                    pools = (pp, den_p, ps_s, ps_y, ps_o)
                    attention_headpair(0, pools)
                    attention_headpair(1, pools, after_g=outproj_g)

            if debug_dumps:
                qt_o = nc.dram_tensor(
                    "qt_o", [128, 2, T], f32, kind="ExternalOutput").ap()
                kt_o = nc.dram_tensor(
                    "kt_o", [128, 2, T], f32, kind="ExternalOutput").ap()
                va_o = nc.dram_tensor(
                    "va_o", [128, HC, NT, 65], f32, kind="ExternalOutput").ap()
                yt_o = nc.dram_tensor(
                    "yt_o", [128, 2, T], f32, kind="ExternalOutput").ap()
                if not skip_q:
                    nc.sync.dma_start(out=qt_o, in_=qt.bitcast(f32))
                if not skip_k:
                    nc.sync.dma_start(out=kt_o, in_=kt.bitcast(f32))
                if not skip_v:
                    nc.sync.dma_start(out=va_o, in_=va.bitcast(f32))
                if stages >= 3:
                    nc.sync.dma_start(out=yt_o, in_=yt.bitcast(f32))

    _split_excess_waits(nc)
    return nc


def kernel(**inputs) -> np.ndarray:
    query = np.ascontiguousarray(np.asarray(inputs["query"], dtype=np.float32))
    Wq = np.asarray(inputs["Wq"], dtype=np.float32)
    Wk = np.asarray(inputs["Wk"], dtype=np.float32)
    Wv = np.asarray(inputs["Wv"], dtype=np.float32)
    Wp = np.asarray(inputs["Wp"], dtype=np.float32)
    bq = np.asarray(inputs["bq"], dtype=np.float32)
    bk = np.asarray(inputs["bk"], dtype=np.float32)
    bv = np.asarray(inputs["bv"], dtype=np.float32)
    bp = np.asarray(inputs["bp"], dtype=np.float32)
    n_head = int(inputs.get("n_head", H))
    assert n_head == H, f"kernel hardcodes n_head={H}, got {n_head}"
    assert query.shape == (B, T, C)

    if "nc" not in _CACHE:
        _CACHE["nc"] = _build_program()
    nc = _CACHE["nc"]

    ones_pv = np.ones((128, 64), np.float32)
    onesrow = np.ones((1, 128), np.float32)
    in_maps = []
    for c in range(8):
        b = c // 4
        hg = c % 4
        cols = slice(DC * hg, DC * (hg + 1))
        in_maps.append({
            "x": query[b],
            "wq": np.ascontiguousarray(Wq[:, cols]),
            "wk": np.ascontiguousarray(Wk[:, cols]),
            "wv": np.ascontiguousarray(Wv[:, cols]),
            "wp": np.ascontiguousarray(Wp[cols, :]),
            "bq": np.ascontiguousarray(bq[cols]),
            "bk": np.ascontiguousarray(bk[cols]),
            "bv": np.ascontiguousarray(bv[cols])[None, :],
            "ones_pv": ones_pv,
            "onesrow": onesrow,
        })

    res = run_bass_kernel_spmd(nc, in_maps, core_ids=list(range(8)))
    _CACHE["last_res"] = res

    out = np.empty((B, T, C), np.float32)
    for b in range(B):
        acc = res.results[4 * b]["out_t"].astype(np.float32)
        for c in range(4 * b + 1, 4 * b + 4):
            acc = acc + res.results[c]["out_t"]
        out[b] = acc.T + bp
    return out



# revision 26
# speedup vs baseline: 1.3359x; 1.3359x over previous
"""Causal self-attention Bass/TRN2 kernel for nn_CausalSelfAttention.

Shapes (hardcoded): query [2, 2048, 1024], 16 heads, d=64.
Sharding: 8 cores = 2 batches x 4 head-groups (4 heads per core, tensor
parallel on QKV/proj weight columns). Each core computes a partial output
projection out_t = Wp_slice^T @ y^T (shape [1024, 2048], bf16); host sums
the 4 partials per batch in f32, transposes, and adds bp.

v2 design (vs the fp32r v1 baseline):
  - X^T is pretransposed on the host and shipped bf16: kills the on-device
    PE transpose stage, its PSUM->SBUF copies, and halves the X DMA bytes.
  - All matmul operands bf16 (PSUM accumulates f32): same PE cost per the
    cost model for N>=256, but removes the fp32r 4x penalty on the narrow
    diagonal S blocks, halves DMA, and shrinks SBUF.
  - Causal diag mask applied ON the PE: an identity x dmask matmul
    accumulated into the S PSUM group, so the S -> exp -> PV chain never
    leaves PE/ACT.
  - Odd heads use va = [ones | v] so PV lands at PSUM partitions 63:128
    (den at 63, y at 64:127): no partition-move DMA for yt[64:128].
  - Coarse DMAs (HWDGE issue is ~630ns of a single shared resource):
    one DMA per weight tensor, one per X^T t-chunk, one per out t-chunk.
  - Software-pipelined emission: S_{j+1}/exp_{j+1} are emitted before PV_j,
    and background work (QK m=1 projections during hp0, out-projection
    during hp1) is drip-fed between attention steps to cover exp latency.

This walrus build accepts only ONE sync-wait command per TPB instruction, so
after Tile scheduling we hoist excess waits into standalone InstEventSemaphore
instructions (split_excess_waits).
"""

from collections import deque

import numpy as np
from ml_dtypes import bfloat16

import concourse.bass as bass
import concourse.mybir as mybir
import concourse.tile as tile
from concourse.bass_utils import run_bass_kernel_spmd

B, T, C, H = 2, 2048, 1024, 16
D = C // H            # 64 head dim
HC = 4                # heads per core
DC = HC * D           # 256 dcols per core
KT = C // 128         # 8 contraction tiles
NT = T // 128         # 16 t-tiles
TCH = T // 512        # 4 t-chunks of 512
SCALE = 1.0 / np.sqrt(D)
NEG = -1.0e30

f32 = mybir.dt.float32
f32r = mybir.dt.float32r
bf16 = mybir.dt.bfloat16

_CACHE = {}


def _split_excess_waits(nc, max_inline=1):
    """Hoist excess per-instruction waits into standalone event-sem waits."""
    n = 0
    for f in nc.m.functions:
        for bb in f.blocks:
            new_insts = []
            for inst in bb.instructions:
                si = inst.sync_info
                waits = list(si.on_wait) if (si is not None and si.on_wait) else []
                if len(waits) > max_inline:
                    hoist, keep = waits[:-max_inline], waits[-max_inline:]
                    for w in hoist:
                        ev = mybir.InstEventSemaphore(
                            name=nc.get_next_instruction_name(),
                            engine=inst.engine,
                            ins=[],
                            outs=[],
                            sync_info=mybir.SyncInfo(on_wait=[w], on_update=[]),
                        )
                        nc.register_instruction(ev, overwrite=True)
                        new_insts.append(ev)
                        n += 1
                    si.on_wait = keep
                new_insts.append(inst)
            bb.instructions[:] = new_insts
    return n


def _make_identity(nc, ident):
    # affine_select KEEPS in_ where the predicate holds and writes `fill`
    # where it does not: identity = fill 1.0 where NOT (p - f != 0).
    nc.gpsimd.memset(ident, 0.0)
    nc.gpsimd.affine_select(
        out=ident, in_=ident, compare_op=mybir.AluOpType.not_equal,
        fill=1.0, base=0, pattern=[[-1, 128]], channel_multiplier=1,
    )


def _make_diag_mask(nc, mask):
    """mask[p, f] = 0 where f >= p (valid, t>=s) else -1e30."""
    nc.gpsimd.memset(mask, 0.0)
    nc.gpsimd.affine_select(
        out=mask, in_=mask, compare_op=mybir.AluOpType.is_ge,
        fill=NEG, base=0, pattern=[[1, 128]], channel_multiplier=-1,
    )


def _build_program():
    nc = bass.Bass("TRN2", target_bir_lowering=False, debug=False)

    xt_d = nc.dram_tensor("xt", [C, T], bf16, kind="ExternalInput").ap()
    wq_d = nc.dram_tensor("wq", [C, DC], bf16, kind="ExternalInput").ap()
    wk_d = nc.dram_tensor("wk", [C, DC], bf16, kind="ExternalInput").ap()
    wv_d = nc.dram_tensor("wv", [C, DC], bf16, kind="ExternalInput").ap()
    wp_d = nc.dram_tensor("wp", [DC, C], bf16, kind="ExternalInput").ap()
    bqk_d = nc.dram_tensor("bqk", [2 * DC], f32, kind="ExternalInput").ap()
    bv_d = nc.dram_tensor("bv", [1, DC], bf16, kind="ExternalInput").ap()
    out_d = nc.dram_tensor("out_t", [C, T], bf16, kind="ExternalOutput").ap()

    with (
        tile.TileContext(nc) as tc,
        nc.allow_low_precision("bf16 storage everywhere; tolerance is 2e-2"),
    ):
        with (
            tc.tile_pool(name="big", bufs=1) as big,
            tc.tile_pool(name="pp", bufs=4) as pp,
            tc.tile_pool(name="rp", bufs=2) as rp,
            tc.tile_pool(name="obp", bufs=2) as obp,
            tc.tile_pool(name="ps_s", bufs=2, space="PSUM") as ps_s,
            tc.tile_pool(name="ps_y", bufs=1, space="PSUM") as ps_y,
            tc.tile_pool(name="ps_m", bufs=2, space="PSUM") as ps_m,
        ):
            # ---- constants ----
            onesb = big.tile([128, 128], bf16)   # all-ones bf16
            nc.gpsimd.memset(onesb, 1.0)
            ones_r = big.tile([128, 64], f32r)   # f32r ones for bc matmuls
            nc.gpsimd.memset(ones_r.bitcast(f32), 1.0)
            identb = big.tile([128, 128], bf16)
            _make_identity(nc, identb)
            dmaskb = big.tile([128, 128], bf16)
            _make_diag_mask(nc, dmaskb)
            bqk_sb = big.tile([128, 2, 2], f32)  # [p, q/k, m]
            bv_sb = big.tile([1, DC], bf16)

            # ---- persistent big tensors ----
            xt = big.tile([128, KT, T], bf16)      # X^T  [c, t]
            wq_sb = big.tile([128, KT, DC], bf16)
            wk_sb = big.tile([128, KT, DC], bf16)
            wv_sb = big.tile([128, KT, DC], bf16)
            wp_sb = big.tile([128, 2, C], bf16)    # [p, m, cout]
            qt = big.tile([128, 2, T], bf16)       # Q^T  [dcol, t]
            kt = big.tile([128, 2, T], bf16)       # K^T
            # V augmented per head: [V_h | ones] (M=65); PV computes y rows
            # 0..63 plus the softmax denominator in row 64.
            va = big.tile([128, HC, NT, 65], bf16)
            yt = big.tile([128, 2, T], bf16)       # normalized y^T

            # ---- input DMAs (coarse; ordered by first use) ----
            def xt_dma(g, ks=slice(0, KT)):
                nc.sync.dma_start(
                    out=xt[:, ks, bass.ts(g, 512)],
                    in_=xt_d[:, bass.ts(g, 512)].rearrange(
                        "(k p) t -> p k t", p=128)[:, ks, :],
                )
            # first QK half-item needs only k0..3 of chunk 0 + wq
            xt_dma(0, slice(0, 4))
            nc.scalar.dma_start(
                out=wq_sb, in_=wq_d.rearrange("(k p) d -> p k d", p=128))
            xt_dma(0, slice(4, KT))
            nc.scalar.dma_start(
                out=wk_sb, in_=wk_d.rearrange("(k p) d -> p k d", p=128))
            nc.scalar.dma_start(
                out=wv_sb, in_=wv_d.rearrange("(k p) d -> p k d", p=128))
            nc.scalar.dma_start(
                out=bqk_sb,
                in_=bqk_d.rearrange("(w m p) -> p w m", p=128, m=2),
            )
            nc.scalar.dma_start(out=bv_sb, in_=bv_d)
            for g in range(1, TCH):
                xt_dma(g)
            nc.scalar.dma_start(
                out=wp_sb, in_=wp_d.rearrange("(m p) t -> p m t", p=128))

            # va ones column (persists; written once)
            nc.vector.memset(va[:, :, :, 64:65], 1.0)

            # ---- helpers ----
            def _one_proj_items(w_sb, dst, wqk, m, g, on_pool):
                """Two ~850ns PE items for one Q^T or K^T chunk (m, g)."""
                st = {}

                def half0():
                    st['p'] = ps_m.tile([128, 512], f32, name="mm")
                    for k in range(4):
                        nc.tensor.matmul(
                            st['p'], w_sb[:, k, bass.ts(m, 128)],
                            xt[:, k, bass.ts(g, 512)],
                            start=(k == 0), stop=False,
                        )

                def half1():
                    p = st['p']
                    for k in range(4, KT):
                        nc.tensor.matmul(
                            p, w_sb[:, k, bass.ts(m, 128)],
                            xt[:, k, bass.ts(g, 512)],
                            start=False, stop=(k == KT - 1),
                        )
                    if on_pool:
                        # (gpsimd cannot read PSUM on hw; DVE does the
                        # bias-add copy when ACT is exp-saturated)
                        nc.vector.tensor_scalar_add(
                            out=dst[:, m, bass.ts(g, 512)], in0=p,
                            scalar1=bqk_sb[:, wqk, m:m + 1],
                        )
                    else:
                        nc.scalar.activation(
                            out=dst[:, m, bass.ts(g, 512)], in_=p,
                            func=mybir.ActivationFunctionType.Identity,
                            bias=bqk_sb[:, wqk, m:m + 1], scale=1.0,
                        )
                return [half0, half1]

            def qk_items(m, g, on_pool):
                return (_one_proj_items(wq_sb, qt, 0, m, g, on_pool)
                        + _one_proj_items(wk_sb, kt, 1, m, g, on_pool))

            def qk_proj(m, g, on_pool=False):
                for it in qk_items(m, g, on_pool):
                    it()

            def v_tile(it):
                """V rows [128it : 128it+128], all 4 heads; +bias via K=1."""
                vp_full = ps_m.tile([128, 512], f32, name="mm")
                vp = vp_full[:, 0:DC]
                for k in range(KT):
                    nc.tensor.matmul(
                        vp, xt[:, k, bass.ts(it, 128)], wv_sb[:, k, :],
                        start=(k == 0), stop=False,
                    )
                nc.tensor.matmul(
                    vp, onesb[0:1, 0:128], bv_sb, start=False, stop=True,
                    skip_group_check=True,
                )
                vh = vp.rearrange("p (h d) -> p h d", d=64)
                nc.vector.tensor_copy(out=va[:, :, it, 0:64], in_=vh)

            # ---- background work queue ----
            bg = deque()

            def emit_bg(n=1):
                for _ in range(n):
                    if bg:
                        bg.popleft()()

            # ---- attention ----
            def s_exp_emit(hp, g, j):
                """S^T for (g, j) both heads -> PSUM; mask; exp -> p12."""
                r = j - 4 * g
                lo = 128 * r if r > 0 else 0
                w = 512 - lo
                s12 = ps_s.tile([128, 1024], f32, name="s12")
                tsl = bass.ds(512 * g + lo, w)
                diag = r >= 0
                nc.tensor.matmul(
                    s12[:, lo:512], kt[0:64, hp, bass.ts(j, 128)],
                    qt[0:64, hp, tsl], start=True, stop=not diag,
                )
                if diag:
                    nc.tensor.matmul(
                        s12[:, lo:lo + 128], identb, dmaskb,
                        start=False, stop=True, skip_group_check=True,
                    )
                nc.tensor.matmul(
                    s12[:, 512 + lo:1024], kt[64:128, hp, bass.ts(j, 128)],
                    qt[64:128, hp, tsl], start=True, stop=not diag,
                )
                if diag:
                    nc.tensor.matmul(
                        s12[:, 512 + lo:512 + lo + 128], identb, dmaskb,
                        start=False, stop=True, skip_group_check=True,
                    )
                p12 = pp.tile([128, 1024], bf16, name="p12")
                sv = s12.rearrange("p (h t) -> p h t", h=2)[:, :, lo:]
                pv = p12.rearrange("p (h t) -> p h t", h=2)[:, :, lo:]
                nc.scalar.activation(
                    out=pv, in_=sv,
                    func=mybir.ActivationFunctionType.Exp,
                    scale=float(SCALE),
                )
                return p12, lo

            def pv_emit(hp, g, j, yd12, p12, lo):
                h1, h2 = 2 * hp, 2 * hp + 1
                nj = 4 * g + 4
                last = j == nj - 1
                nc.tensor.matmul(
                    yd12[0:65, lo:512], va[:, h1 % 4, j, :],
                    p12[:, lo:512], start=(j == 0), stop=last,
                    skip_group_check=True,
                )
                nc.tensor.matmul(
                    yd12[0:65, 512 + lo:1024], va[:, h2 % 4, j, :],
                    p12[:, 512 + lo:1024], start=(j == 0), stop=last,
                    skip_group_check=True,
                )

            def normalize_emit(hp, g, yd12):
                # 1/den for both heads at partition 64 (row group (64,0)),
                # broadcast to 64 rows via one K=1 ones matmul; the muls may
                # read only ONE PSUM operand, so stage the broadcast back to
                # SBUF (split across ACT and DVE so the chain is short).
                r12 = rp.tile([128, 1024], f32r, name="r12")
                nc.vector.reciprocal(out=r12[64:65, :], in_=yd12[64:65, :])
                bc12 = ps_s.tile([128, 1024], f32, name="s12")
                nc.tensor.matmul(
                    bc12[0:64, 0:512], ones_r[64:65, :], r12[64:65, 0:512],
                    start=True, stop=True,
                )
                nc.tensor.matmul(
                    bc12[0:64, 512:1024], ones_r[64:65, :],
                    r12[64:65, 512:1024], start=True, stop=True,
                )
                rb = rp.tile([64, 1024], bf16, name="rb")
                nc.scalar.activation(
                    out=rb[:, 512:1024], in_=bc12[0:64, 512:1024],
                    func=mybir.ActivationFunctionType.Copy,
                )
                nc.vector.tensor_copy(out=rb[:, 0:512], in_=bc12[0:64, 0:512])
                # odd head first: its partition-move DMA is latency-bound
                ytmp = rp.tile([64, 512], bf16, name="ytmp")
                nc.vector.tensor_mul(
                    ytmp, yd12[0:64, 512:1024], rb[:, 512:1024])
                nc.sync.dma_start(
                    out=yt[64:128, hp, bass.ts(g, 512)], in_=ytmp)
                nc.vector.tensor_mul(
                    yt[0:64, hp, bass.ts(g, 512)], yd12[0:64, 0:512],
                    rb[:, 0:512])

            def attention(chunks, after_chunk=None):
                """chunks: ordered list of (hp, g); software-pipelined with
                lookahead-1 across chunk (and head-pair) boundaries."""
                steps = [(hp, g, j)
                         for hp, g in chunks for j in range(4 * g + 4)]
                yd = {}

                def get_yd(hp, g):
                    if (hp, g) not in yd:
                        yd[hp, g] = ps_y.tile([128, 1024], f32, name="yd12")
                    return yd[hp, g]

                pend = {}  # (hp, g, j) -> (p12, lo)
                pend[steps[0]] = s_exp_emit(*steps[0])
                for i, (hp, g, j) in enumerate(steps):
                    if i + 1 < len(steps):
                        pend[steps[i + 1]] = s_exp_emit(*steps[i + 1])
                    # background PE work goes BEFORE PV_j: PV_j blocks the PE
                    # queue on exp_j, the bg item fills that latency.
                    emit_bg(1)
                    yd12 = get_yd(hp, g)
                    p12, lo = pend.pop((hp, g, j))
                    pv_emit(hp, g, j, yd12, p12, lo)
                    if j == 4 * g + 3:  # last j of chunk g
                        emit_bg(1)
                        normalize_emit(hp, g, yd12)
                        del yd[hp, g]
                        if after_chunk is not None:
                            after_chunk(hp, g)

            # ---- out projection (hp1 background) ----
            def proj_mo(g, mo, ob):
                op = ps_m.tile([128, 512], f32, name="mm")
                for m in range(2):
                    nc.tensor.matmul(
                        op, wp_sb[:, m, bass.ts(mo, 128)],
                        yt[:, m, bass.ts(g, 512)],
                        start=(m == 0), stop=(m == 1),
                    )
                nc.vector.tensor_copy(out=ob[:, mo % 2, :], in_=op)

            def proj_g(g, via_bg=True):
                """Projection for chunk g: 4 mo-pairs, each staged to SBUF
                bf16 and DMA'd independently (short tail)."""
                items = []
                for mp in range(4):
                    ob = obp.tile([128, 2, 512], bf16, name="ob")
                    items.append(lambda mo=2 * mp, ob=ob: proj_mo(g, mo, ob))
                    items.append(
                        lambda mo=2 * mp + 1, ob=ob: proj_mo(g, mo, ob))

                    def dma_item(mp=mp, ob=ob):
                        nc.sync.dma_start(
                            out=out_d[bass.ts(mp, 256), bass.ts(g, 512)]
                            .rearrange("(mo p) t -> p mo t", p=128),
                            in_=ob,
                        )
                    items.append(dma_item)
                if via_bg:
                    bg.extend(items)
                else:
                    for it in items:
                        it()

            # ================= emission =================
            # Prelude: everything the first chunks depend on up front
            # (QK m=0 all chunks, V blocks 0..3); the rest drips in as bg.
            qk_proj(0, 0)
            for it in range(4):
                v_tile(it)
            for g in range(1, TCH):
                qk_proj(0, g)

            # Background: remaining V blocks and QK m=1 (bias on Pool: ACT
            # is exp-saturated during attention), ordered well before the
            # attention steps that consume them.
            for it in range(4, 8):
                bg.append(lambda it=it: v_tile(it))
            for g in range(2):
                bg.extend(qk_items(1, g, on_pool=True))
            for it in range(8, 12):
                bg.append(lambda it=it: v_tile(it))
            for g in range(2, TCH):
                bg.extend(qk_items(1, g, on_pool=True))
            for it in range(12, NT):
                bg.append(lambda it=it: v_tile(it))

            # Interleave head-pair chunks so projection work (ready once
            # both head-pairs of a t-chunk are normalized) spreads across
            # the remaining attention instead of piling up at the end.
            chunks = [(0, 0), (0, 1), (1, 0), (0, 2), (1, 1), (0, 3),
                      (1, 2), (1, 3)]
            done = set()

            def after_chunk(hp, g):
                done.add((hp, g))
                if (0, g) in done and (1, g) in done:
                    proj_g(g, via_bg=(hp, g) != chunks[-1])

            attention(chunks, after_chunk=after_chunk)
            while bg:
                bg.popleft()()

    _split_excess_waits(nc)
    return nc


def kernel(**inputs) -> np.ndarray:
    query = np.asarray(inputs["query"], dtype=np.float32)
    Wq = np.asarray(inputs["Wq"], dtype=np.float32)
    Wk = np.asarray(inputs["Wk"], dtype=np.float32)
    Wv = np.asarray(inputs["Wv"], dtype=np.float32)
    Wp = np.asarray(inputs["Wp"], dtype=np.float32)
    bq = np.asarray(inputs["bq"], dtype=np.float32)
    bk = np.asarray(inputs["bk"], dtype=np.float32)
    bv = np.asarray(inputs["bv"], dtype=np.float32)
    bp = np.asarray(inputs["bp"], dtype=np.float32)
    n_head = int(inputs.get("n_head", H))
    assert n_head == H, f"kernel hardcodes n_head={H}, got {n_head}"
    assert query.shape == (B, T, C)

    if "nc" not in _CACHE:
        _CACHE["nc"] = _build_program()
    nc = _CACHE["nc"]

    xts = [
        np.ascontiguousarray(query[b].T).astype(bfloat16) for b in range(B)
    ]
    in_maps = []
    for c in range(8):
        b = c // 4
        hg = c % 4
        cols = slice(DC * hg, DC * (hg + 1))
        in_maps.append({
            "xt": xts[b],
            "wq": np.ascontiguousarray(Wq[:, cols]).astype(bfloat16),
            "wk": np.ascontiguousarray(Wk[:, cols]).astype(bfloat16),
            "wv": np.ascontiguousarray(Wv[:, cols]).astype(bfloat16),
            "wp": np.ascontiguousarray(Wp[cols, :]).astype(bfloat16),
            "bqk": np.concatenate([bq[cols], bk[cols]]).astype(np.float32),
            "bv": np.ascontiguousarray(bv[cols])[None, :].astype(bfloat16),
        })

    res = run_bass_kernel_spmd(nc, in_maps, core_ids=list(range(8)))
    _CACHE["last_res"] = res

    out = np.empty((B, T, C), np.float32)
    for b in range(B):
        acc = res.results[4 * b]["out_t"].astype(np.float32)
        for c in range(4 * b + 1, 4 * b + 4):
            acc = acc + res.results[c]["out_t"].astype(np.float32)
        out[b] = acc.T + bp
    return out


# revision 29
# speedup vs baseline: 1.4471x; 1.0833x over previous
"""Causal self-attention Bass/TRN2 kernel for nn_CausalSelfAttention.

Shapes (hardcoded): query [2, 2048, 1024], 16 heads, d=64.
Sharding: 8 cores = 2 batches x 4 head-groups (4 heads per core, tensor
parallel on QKV/proj weight columns). Each core computes a partial output
projection out_t = Wp_slice^T @ y^T (shape [1024, 2048], bf16); host sums
the 4 partials per batch in f32, transposes, and adds bp.

v2 design (vs the fp32r v1 baseline):
  - X^T is pretransposed on the host and shipped bf16: kills the on-device
    PE transpose stage, its PSUM->SBUF copies, and halves the X DMA bytes.
  - All matmul operands bf16 (PSUM accumulates f32): same PE cost per the
    cost model for N>=256, but removes the fp32r 4x penalty on the narrow
    diagonal S blocks, halves DMA, and shrinks SBUF.
  - Causal diag mask applied ON the PE: an identity x dmask matmul
    accumulated into the S PSUM group, so the S -> exp -> PV chain never
    leaves PE/ACT.
  - Odd heads use va = [ones | v] so PV lands at PSUM partitions 63:128
    (den at 63, y at 64:127): no partition-move DMA for yt[64:128].
  - Coarse DMAs (HWDGE issue is ~630ns of a single shared resource):
    one DMA per weight tensor, one per X^T t-chunk, one per out t-chunk.
  - Software-pipelined emission: S_{j+1}/exp_{j+1} are emitted before PV_j,
    and background work (QK m=1 projections during hp0, out-projection
    during hp1) is drip-fed between attention steps to cover exp latency.

This walrus build accepts only ONE sync-wait command per TPB instruction, so
after Tile scheduling we hoist excess waits into standalone InstEventSemaphore
instructions (split_excess_waits).
"""

from collections import deque

import numpy as np
from ml_dtypes import bfloat16

import concourse.bass as bass
import concourse.mybir as mybir
import concourse.tile as tile
from concourse.bass_utils import run_bass_kernel_spmd

B, T, C, H = 2, 2048, 1024, 16
D = C // H            # 64 head dim
HC = 4                # heads per core
DC = HC * D           # 256 dcols per core
KT = C // 128         # 8 contraction tiles
NT = T // 128         # 16 t-tiles
TCH = T // 512        # 4 t-chunks of 512
SCALE = 1.0 / np.sqrt(D)
NEG = -1.0e30

f32 = mybir.dt.float32
f32r = mybir.dt.float32r
bf16 = mybir.dt.bfloat16

_CACHE = {}


def _split_excess_waits(nc, max_inline=1):
    """Hoist excess per-instruction waits into standalone event-sem waits."""
    n = 0
    for f in nc.m.functions:
        for bb in f.blocks:
            new_insts = []
            for inst in bb.instructions:
                si = inst.sync_info
                waits = list(si.on_wait) if (si is not None and si.on_wait) else []
                if len(waits) > max_inline:
                    hoist, keep = waits[:-max_inline], waits[-max_inline:]
                    for w in hoist:
                        ev = mybir.InstEventSemaphore(
                            name=nc.get_next_instruction_name(),
                            engine=inst.engine,
                            ins=[],
                            outs=[],
                            sync_info=mybir.SyncInfo(on_wait=[w], on_update=[]),
                        )
                        nc.register_instruction(ev, overwrite=True)
                        new_insts.append(ev)
                        n += 1
                    si.on_wait = keep
                new_insts.append(inst)
            bb.instructions[:] = new_insts
    return n


def _make_identity(nc, ident):
    # affine_select KEEPS in_ where the predicate holds and writes `fill`
    # where it does not: identity = fill 1.0 where NOT (p - f != 0).
    nc.gpsimd.memset(ident, 0.0)
    nc.gpsimd.affine_select(
        out=ident, in_=ident, compare_op=mybir.AluOpType.not_equal,
        fill=1.0, base=0, pattern=[[-1, 128]], channel_multiplier=1,
    )


def _make_diag_mask(nc, mask):
    """mask[p, f] = 0 where f >= p (valid, t>=s) else -1e30."""
    nc.gpsimd.memset(mask, 0.0)
    nc.gpsimd.affine_select(
        out=mask, in_=mask, compare_op=mybir.AluOpType.is_ge,
        fill=NEG, base=0, pattern=[[1, 128]], channel_multiplier=-1,
    )


def _build_program(with_bias=True):
    nc = bass.Bass("TRN2", target_bir_lowering=False, debug=False)

    xt_d = nc.dram_tensor("xt", [C, T], bf16, kind="ExternalInput").ap()
    wq_d = nc.dram_tensor("wq", [C, DC], bf16, kind="ExternalInput").ap()
    wk_d = nc.dram_tensor("wk", [C, DC], bf16, kind="ExternalInput").ap()
    wv_d = nc.dram_tensor("wv", [C, DC], bf16, kind="ExternalInput").ap()
    wp_d = nc.dram_tensor("wp", [DC, C], bf16, kind="ExternalInput").ap()
    bqk_d = nc.dram_tensor("bqk", [2 * DC], f32, kind="ExternalInput").ap()
    bv_d = nc.dram_tensor("bv", [1, DC], bf16, kind="ExternalInput").ap()
    out_d = nc.dram_tensor("out_t", [C, T], bf16, kind="ExternalOutput").ap()

    with (
        tile.TileContext(nc) as tc,
        nc.allow_low_precision("bf16 storage everywhere; tolerance is 2e-2"),
    ):
        with (
            tc.tile_pool(name="big", bufs=1) as big,
            tc.tile_pool(name="pp", bufs=4) as pp,
            tc.tile_pool(name="rp", bufs=2) as rp,
            tc.tile_pool(name="obp", bufs=2) as obp,
            tc.tile_pool(name="ps_s", bufs=2, space="PSUM") as ps_s,
            tc.tile_pool(name="ps_y", bufs=1, space="PSUM") as ps_y,
            tc.tile_pool(name="ps_m", bufs=2, space="PSUM") as ps_m,
        ):
            # ---- constants ----
            onesb = big.tile([128, 128], bf16)   # all-ones bf16
            nc.gpsimd.memset(onesb, 1.0)
            ones_r = big.tile([128, 64], f32r)   # f32r ones for bc matmuls
            nc.gpsimd.memset(ones_r.bitcast(f32), 1.0)
            identb = big.tile([128, 128], bf16)
            _make_identity(nc, identb)
            dmaskb = big.tile([128, 128], bf16)
            _make_diag_mask(nc, dmaskb)
            bqk_sb = big.tile([128, 2, 2], f32)  # [p, q/k, m]
            bv_sb = big.tile([1, DC], bf16)

            # ---- persistent big tensors ----
            xt = big.tile([128, KT, T], bf16)      # X^T  [c, t]
            wq_sb = big.tile([128, KT, DC], bf16)
            wk_sb = big.tile([128, KT, DC], bf16)
            wv_sb = big.tile([128, KT, DC], bf16)
            wp_sb = big.tile([128, 2, C], bf16)    # [p, m, cout]
            qt = big.tile([128, 2, T], bf16)       # Q^T  [dcol, t]
            kt = big.tile([128, 2, T], bf16)       # K^T
            # V augmented per head: [V_h | ones] (M=65); PV computes y rows
            # 0..63 plus the softmax denominator in row 64.
            va = big.tile([128, HC, NT, 65], bf16)
            yt = big.tile([128, 2, T], bf16)       # normalized y^T

            # ---- input DMAs (coarse; ordered by first use) ----
            def xt_dma(g, ks=slice(0, KT)):
                nc.sync.dma_start(
                    out=xt[:, ks, bass.ts(g, 512)],
                    in_=xt_d[:, bass.ts(g, 512)].rearrange(
                        "(k p) t -> p k t", p=128)[:, ks, :],
                )
            # first QK half-item needs only k0..3 of chunk 0 + wq
            xt_dma(0, slice(0, 4))
            nc.scalar.dma_start(
                out=wq_sb, in_=wq_d.rearrange("(k p) d -> p k d", p=128))
            xt_dma(0, slice(4, KT))
            nc.scalar.dma_start(
                out=wk_sb, in_=wk_d.rearrange("(k p) d -> p k d", p=128))
            nc.scalar.dma_start(
                out=wv_sb, in_=wv_d.rearrange("(k p) d -> p k d", p=128))
            if with_bias:
                nc.scalar.dma_start(
                    out=bqk_sb,
                    in_=bqk_d.rearrange("(w m p) -> p w m", p=128, m=2),
                )
                nc.scalar.dma_start(out=bv_sb, in_=bv_d)
            for g in range(1, TCH):
                xt_dma(g)
            nc.scalar.dma_start(
                out=wp_sb, in_=wp_d.rearrange("(m p) t -> p m t", p=128))

            # va ones column (persists; written once)
            nc.vector.memset(va[:, :, :, 64:65], 1.0)

            # ---- helpers ----
            def _one_proj_items(w_sb, dst, wqk, m, g, on_pool):
                """Two ~850ns PE items for one Q^T or K^T chunk (m, g)."""
                st = {}

                def half0():
                    st['p'] = ps_m.tile([128, 512], f32, name="mm")
                    for k in range(4):
                        nc.tensor.matmul(
                            st['p'], w_sb[:, k, bass.ts(m, 128)],
                            xt[:, k, bass.ts(g, 512)],
                            start=(k == 0), stop=False,
                        )

                def half1():
                    p = st['p']
                    for k in range(4, KT):
                        nc.tensor.matmul(
                            p, w_sb[:, k, bass.ts(m, 128)],
                            xt[:, k, bass.ts(g, 512)],
                            start=False, stop=(k == KT - 1),
                        )
                    if not with_bias:
                        eng = nc.vector if on_pool else nc.scalar
                        if on_pool:
                            nc.vector.tensor_copy(
                                out=dst[:, m, bass.ts(g, 512)], in_=p)
                        else:
                            nc.scalar.activation(
                                out=dst[:, m, bass.ts(g, 512)], in_=p,
                                func=mybir.ActivationFunctionType.Copy,
                            )
                    elif on_pool:
                        # (gpsimd cannot read PSUM on hw; DVE does the
                        # bias-add copy when ACT is exp-saturated)
                        nc.vector.tensor_scalar_add(
                            out=dst[:, m, bass.ts(g, 512)], in0=p,
                            scalar1=bqk_sb[:, wqk, m:m + 1],
                        )
                    else:
                        nc.scalar.activation(
                            out=dst[:, m, bass.ts(g, 512)], in_=p,
                            func=mybir.ActivationFunctionType.Identity,
                            bias=bqk_sb[:, wqk, m:m + 1], scale=1.0,
                        )
                return [half0, half1]

            def qk_items(m, g, on_pool):
                return (_one_proj_items(wq_sb, qt, 0, m, g, on_pool)
                        + _one_proj_items(wk_sb, kt, 1, m, g, on_pool))

            def qk_proj(m, g, on_pool=False):
                for it in qk_items(m, g, on_pool):
                    it()

            def v_tile(it):
                """V rows [128it : 128it+128], all 4 heads; +bias via K=1."""
                vp_full = ps_m.tile([128, 512], f32, name="mm")
                vp = vp_full[:, 0:DC]
                for k in range(KT):
                    nc.tensor.matmul(
                        vp, xt[:, k, bass.ts(it, 128)], wv_sb[:, k, :],
                        start=(k == 0), stop=(k == KT - 1 and not with_bias),
                    )
                if with_bias:
                    nc.tensor.matmul(
                        vp, onesb[0:1, 0:128], bv_sb, start=False, stop=True,
                        skip_group_check=True,
                    )
                vh = vp.rearrange("p (h d) -> p h d", d=64)
                nc.vector.tensor_copy(out=va[:, :, it, 0:64], in_=vh)

            # ---- background work queue (dependency-keyed) ----
            bg = deque()          # items: (key or None, fn)
            provided = set()      # provider keys already emitted

            def bg_add(key, fn):
                bg.append((key, fn))

            def emit_bg(n=1):
                for _ in range(n):
                    if bg:
                        key, fn = bg.popleft()
                        fn()
                        if key is not None:
                            provided.add(key)

            def require(*keys):
                while any(k not in provided for k in keys):
                    assert bg, f"missing bg providers for {keys}"
                    emit_bg(1)

            # ---- attention ----
            def s_exp_emit(hp, g, j):
                """S^T for (g, j) both heads -> PSUM; mask; exp -> p12."""
                r = j - 4 * g
                lo = 128 * r if r > 0 else 0
                w = 512 - lo
                s12 = ps_s.tile([128, 1024], f32, name="s12")
                tsl = bass.ds(512 * g + lo, w)
                diag = r >= 0
                nc.tensor.matmul(
                    s12[:, lo:512], kt[0:64, hp, bass.ts(j, 128)],
                    qt[0:64, hp, tsl], start=True, stop=not diag,
                )
                if diag:
                    nc.tensor.matmul(
                        s12[:, lo:lo + 128], identb, dmaskb,
                        start=False, stop=True, skip_group_check=True,
                    )
                nc.tensor.matmul(
                    s12[:, 512 + lo:1024], kt[64:128, hp, bass.ts(j, 128)],
                    qt[64:128, hp, tsl], start=True, stop=not diag,
                )
                if diag:
                    nc.tensor.matmul(
                        s12[:, 512 + lo:512 + lo + 128], identb, dmaskb,
                        start=False, stop=True, skip_group_check=True,
                    )
                p12 = pp.tile([128, 1024], bf16, name="p12")
                sv = s12.rearrange("p (h t) -> p h t", h=2)[:, :, lo:]
                pv = p12.rearrange("p (h t) -> p h t", h=2)[:, :, lo:]
                nc.scalar.activation(
                    out=pv, in_=sv,
                    func=mybir.ActivationFunctionType.Exp,
                    scale=float(SCALE),
                )
                return p12, lo

            def pv_emit(hp, g, j, yd12, p12, lo):
                h1, h2 = 2 * hp, 2 * hp + 1
                nj = 4 * g + 4
                last = j == nj - 1
                nc.tensor.matmul(
                    yd12[0:65, lo:512], va[:, h1 % 4, j, :],
                    p12[:, lo:512], start=(j == 0), stop=last,
                    skip_group_check=True,
                )
                nc.tensor.matmul(
                    yd12[0:65, 512 + lo:1024], va[:, h2 % 4, j, :],
                    p12[:, 512 + lo:1024], start=(j == 0), stop=last,
                    skip_group_check=True,
                )

            def normalize_emit(hp, g, yd12, last=False):
                # 1/den for both heads at partition 64 (row group (64,0)),
                # broadcast to 64 rows via one K=1 ones matmul; the muls may
                # read only ONE PSUM operand, so stage the broadcast back to
                # SBUF (split across ACT and DVE so the chain is short).
                r12 = rp.tile([128, 1024], f32r, name="r12")
                nc.vector.reciprocal(out=r12[64:65, :], in_=yd12[64:65, :])
                bc12 = ps_s.tile([128, 1024], f32, name="s12")
                nc.tensor.matmul(
                    bc12[0:64, 0:512], ones_r[64:65, :], r12[64:65, 0:512],
                    start=True, stop=True,
                )
                nc.tensor.matmul(
                    bc12[0:64, 512:1024], ones_r[64:65, :],
                    r12[64:65, 512:1024], start=True, stop=True,
                )
                rb = rp.tile([64, 1024], bf16, name="rb")
                nc.scalar.activation(
                    out=rb[:, 512:1024], in_=bc12[0:64, 512:1024],
                    func=mybir.ActivationFunctionType.Copy,
                )
                nc.vector.tensor_copy(out=rb[:, 0:512], in_=bc12[0:64, 0:512])
                # odd head first: its partition-move DMA is latency-bound
                ytmp = rp.tile([64, 512], bf16, name="ytmp")
                nc.vector.tensor_mul(
                    ytmp, yd12[0:64, 512:1024], rb[:, 512:1024])
                if last:
                    # tail: PE identity-move + DVE copy beats DMA latency
                    mv = ps_m.tile([128, 512], f32, name="mm")
                    nc.tensor.matmul(
                        mv[64:128, :], identb[0:64, 0:64], ytmp,
                        start=True, stop=True,
                    )
                    nc.vector.tensor_copy(
                        out=yt[64:128, hp, bass.ts(g, 512)], in_=mv[64:128, :])
                else:
                    nc.sync.dma_start(
                        out=yt[64:128, hp, bass.ts(g, 512)], in_=ytmp)
                nc.vector.tensor_mul(
                    yt[0:64, hp, bass.ts(g, 512)], yd12[0:64, 0:512],
                    rb[:, 0:512])

            def attention(chunks, after_chunk=None):
                """chunks: ordered list of (hp, g); software-pipelined with
                lookahead-1 across chunk (and head-pair) boundaries."""
                steps = [(hp, g, j)
                         for hp, g in chunks for j in range(4 * g + 4)]
                yd = {}

                def get_yd(hp, g):
                    if (hp, g) not in yd:
                        yd[hp, g] = ps_y.tile([128, 1024], f32, name="yd12")
                    return yd[hp, g]

                pend = {}  # (hp, g, j) -> (p12, lo)
                emitted = [0]

                def ensure(k):
                    # software-pipeline cursor: s_exp for steps <= k emitted
                    while emitted[0] <= min(k, len(steps) - 1):
                        s = steps[emitted[0]]
                        shp, sg, sj = s
                        require(('qk', shp, sg), ('qk', shp, sj // 4))
                        pend[s] = s_exp_emit(*s)
                        emitted[0] += 1

                ensure(0)
                for i, (hp, g, j) in enumerate(steps):
                    ensure(i + 1)
                    # background PE work goes BEFORE PV_j: PV_j blocks the PE
                    # queue on exp_j, the bg item fills that latency.
                    emit_bg(1)
                    require(('v', j))
                    yd12 = get_yd(hp, g)
                    p12, lo = pend.pop((hp, g, j))
                    pv_emit(hp, g, j, yd12, p12, lo)
                    if j == 4 * g + 3:  # last j of chunk g
                        # deeper lookahead over the boundary so the PE isn't
                        # starved while the normalize chain runs
                        ensure(i + 2)
                        emit_bg(1)
                        normalize_emit(hp, g, yd12, last=(i == len(steps) - 1))
                        del yd[hp, g]
                        if after_chunk is not None:
                            after_chunk(hp, g)

            # ---- out projection (hp1 background) ----
            def proj_mo(g, mo, ob):
                op = ps_m.tile([128, 512], f32, name="mm")
                for m in range(2):
                    nc.tensor.matmul(
                        op, wp_sb[:, m, bass.ts(mo, 128)],
                        yt[:, m, bass.ts(g, 512)],
                        start=(m == 0), stop=(m == 1),
                    )
                nc.vector.tensor_copy(out=ob[:, mo % 2, :], in_=op)

            def proj_g(g, via_bg=True):
                """Projection for chunk g: 4 mo-pairs, each staged to SBUF
                bf16 and DMA'd independently (short tail)."""
                items = []
                for mp in range(4):
                    ob = obp.tile([128, 2, 512], bf16, name="ob")
                    items.append(lambda mo=2 * mp, ob=ob: proj_mo(g, mo, ob))
                    items.append(
                        lambda mo=2 * mp + 1, ob=ob: proj_mo(g, mo, ob))

                    def dma_item(mp=mp, ob=ob):
                        nc.sync.dma_start(
                            out=out_d[bass.ts(mp, 256), bass.ts(g, 512)]
                            .rearrange("(mo p) t -> p mo t", p=128),
                            in_=ob,
                        )
                    items.append(dma_item)
                if via_bg:
                    for it in items:
                        bg_add(None, it)
                else:
                    for it in items:
                        it()

            # ================= emission =================
            # Prelude: only the first QK chunk; everything else drips in as
            # keyed background work pulled just-in-time by require().
            qk_proj(0, 0)
            provided.add(('qk', 0, 0))

            def add_v(lo_it, hi_it):
                for it in range(lo_it, hi_it):
                    bg_add(('v', it), lambda it=it: v_tile(it))

            def add_qk(m, g, on_pool):
                its = qk_items(m, g, on_pool)
                for f in its[:-1]:
                    bg_add(None, f)
                bg_add(('qk', m, g), its[-1])

            add_v(0, 4)
            add_qk(0, 1, False)
            add_v(4, 8)
            add_qk(1, 0, True)
            add_qk(1, 1, True)
            add_qk(0, 2, False)
            add_v(8, 12)
            add_qk(1, 2, True)
            add_qk(0, 3, True)
            add_v(12, NT)
            add_qk(1, 3, True)

            # Interleave head-pair chunks so projection work (ready once
            # both head-pairs of a t-chunk are normalized) spreads across
            # the remaining attention instead of piling up at the end.
            chunks = [(0, 0), (0, 1), (1, 0), (0, 2), (1, 1), (0, 3),
                      (1, 2), (1, 3)]
            done = set()

            def after_chunk(hp, g):
                done.add((hp, g))
                if (0, g) in done and (1, g) in done:
                    proj_g(g, via_bg=(hp, g) != chunks[-1])

            attention(chunks, after_chunk=after_chunk)
            while bg:
                bg.popleft()()

    _split_excess_waits(nc)
    return nc


def kernel(**inputs) -> np.ndarray:
    query = np.asarray(inputs["query"], dtype=np.float32)
    Wq = np.asarray(inputs["Wq"], dtype=np.float32)
    Wk = np.asarray(inputs["Wk"], dtype=np.float32)
    Wv = np.asarray(inputs["Wv"], dtype=np.float32)
    Wp = np.asarray(inputs["Wp"], dtype=np.float32)
    bq = np.asarray(inputs["bq"], dtype=np.float32)
    bk = np.asarray(inputs["bk"], dtype=np.float32)
    bv = np.asarray(inputs["bv"], dtype=np.float32)
    bp = np.asarray(inputs["bp"], dtype=np.float32)
    n_head = int(inputs.get("n_head", H))
    assert n_head == H, f"kernel hardcodes n_head={H}, got {n_head}"
    assert query.shape == (B, T, C)

    with_bias = bool(np.any(bq) or np.any(bk) or np.any(bv))
    key = ("nc", with_bias)
    if key not in _CACHE:
        _CACHE[key] = _build_program(with_bias=with_bias)
    nc = _CACHE[key]

    xts = [
        np.ascontiguousarray(query[b].T).astype(bfloat16) for b in range(B)
    ]
    in_maps = []
    for c in range(8):
        b = c // 4
        hg = c % 4
        cols = slice(DC * hg, DC * (hg + 1))
        in_maps.append({
            "xt": xts[b],
            "wq": np.ascontiguousarray(Wq[:, cols]).astype(bfloat16),
            "wk": np.ascontiguousarray(Wk[:, cols]).astype(bfloat16),
            "wv": np.ascontiguousarray(Wv[:, cols]).astype(bfloat16),
            "wp": np.ascontiguousarray(Wp[cols, :]).astype(bfloat16),
            "bqk": np.concatenate([bq[cols], bk[cols]]).astype(np.float32),
            "bv": np.ascontiguousarray(bv[cols])[None, :].astype(bfloat16),
        })

    res = run_bass_kernel_spmd(nc, in_maps, core_ids=list(range(8)))
    _CACHE["last_res"] = res

    out = np.empty((B, T, C), np.float32)
    for b in range(B):
        acc = res.results[4 * b]["out_t"].astype(np.float32)
        for c in range(4 * b + 1, 4 * b + 4):
            acc = acc + res.results[c]["out_t"].astype(np.float32)
        out[b] = acc.T + bp
    return out
